# revision 48
# baseline (speedup 1.0000x reference)
"""Causal self-attention with relative position bias, 8-way batch-parallel
across NeuronCores. Self-contained: takes full inputs, returns full output.

Per-core layout strategy:
  - host feeds transposed weights (c-major) so every matmul operand already
    has the contraction dim in partitions; no transpose loads on device
  - rel bias: S2 = q @ R_rev^T per head, then a sheared (diagonal)
    SBUF->SBUF DMA materializes rel[i, j] = S2[i, i-j]; pad columns pre-set
    to -1e30 make the causal mask fall out of exp() for free
  - scores computed TRANSPOSED per 128-block: scT[k, q] = k.q via swapped
    matmul operands; rel^T accumulated into the same PSUM region by a second
    matmul (lhsT=rel block, rhs=identity). exp() then writes att^T straight
    to SBUF -- no PE transposes of att, no vector-engine evictions of it
  - softmax denominator from a ones-column appended to v (row-sum rides the
    AV matmul); no max-subtraction (scores bounded well inside fp32 range)
  - q/k/proj biases enter their PSUM accumulations as K=1 rank-1 matmuls,
    so bn_stats reads the qkv PSUM directly and evictions are pure copies
  - qkv is stream-major (q, then k, then v) with per-stream DVE-only Newton
    rsqrt (single ACT LUT table): q's layernorm chain overlaps k's matmuls
  - the y-transpose + output projection of rep r are issued during rep
    r+1's layernorm window (PE filler); heads are software-pipelined 2 deep
    with the first two heads' S2/shear chains launched mid-layernorm
  - elementwise work is split across ACT (activation scale/bias) and DVE
    (tensor_scalar with per-partition scalar pointers); shear DMA issue cost
    alternates between the gpsimd and sync queues
  - bf16 throughout the matmul paths (validated ~4e-3 scale-relative error)
"""

import numpy as np

B, T, C, NH = 8, 512, 512, 8
HD = C // NH  # 64
N_CORES = 8
EPS = 1e-5
NEG = -1.0e30

_CACHE = {}


def _build_nc(reps: int = 1, pools=(2, 3, 2, 1), ln_bufs=3):
    from contextlib import ExitStack

    import concourse.bass as bass
    import concourse.mybir as mybir
    import concourse.tile as tile
    from concourse import bacc
    from concourse.masks import make_identity

    f32 = mybir.dt.float32
    bf16 = mybir.dt.bfloat16
    Alu = mybir.AluOpType
    Act = mybir.ActivationFunctionType

    nc = bacc.Bacc("TRN2")

    xT = nc.declare_dram_parameter("xT", [C, T], bf16, isOutput=False)
    w_attnT = nc.declare_dram_parameter("w_attnT", [C, 3 * C], bf16, isOutput=False)
    b_attn = nc.declare_dram_parameter("b_attn", [1, 3 * C], f32, isOutput=False)
    b_attn_bf = nc.declare_dram_parameter("b_attn_bf", [1, 3 * C], bf16, isOutput=False)
    b_proj_bf = nc.declare_dram_parameter("b_proj_bf", [1, C], bf16, isOutput=False)
    w_projT = nc.declare_dram_parameter("w_projT", [C, C], bf16, isOutput=False)
    b_proj = nc.declare_dram_parameter("b_proj", [1, C], f32, isOutput=False)
    gbT = nc.declare_dram_parameter("gbT", [C, 4], f32, isOutput=False)
    rT_rev = nc.declare_dram_parameter("rT_rev", [C, T], bf16, isOutput=False)
    y = nc.declare_dram_parameter("y", [T, C], f32, isOutput=True)

    with tile.TileContext(nc) as tc, ExitStack() as ctx:
        const = ctx.enter_context(tc.tile_pool(name="const", bufs=1))
        qk_nat = ctx.enter_context(tc.tile_pool(name="qk_nat", bufs=4))
        qkT = ctx.enter_context(tc.tile_pool(name="qkT", bufs=2))
        stats = ctx.enter_context(tc.tile_pool(name="stats", bufs=2))
        rel_pool = ctx.enter_context(tc.tile_pool(name="rel_pool", bufs=4))
        attT_pool = ctx.enter_context(tc.tile_pool(name="attT_pool", bufs=4))
        y_pool = ctx.enter_context(tc.tile_pool(name="y_pool", bufs=2))
        out_pool = ctx.enter_context(tc.tile_pool(name="out_pool", bufs=2))
        s2_ps = ctx.enter_context(tc.tile_pool(name="s2_ps", bufs=pools[0], space="PSUM"))
        sc_ps = ctx.enter_context(tc.tile_pool(name="sc_ps", bufs=pools[1], space="PSUM"))
        tp_ps = ctx.enter_context(tc.tile_pool(name="tp_ps", bufs=pools[2], space="PSUM"))
        av_ps = ctx.enter_context(tc.tile_pool(name="av_ps", bufs=pools[3], space="PSUM"))

        # ---- constant loads ------------------------------------------------
        xT_d = xT.rearrange("(ct p) t -> p ct t", p=128)
        waT_d = w_attnT.rearrange("(ct p) t -> p ct t", p=128)
        xT_sb = const.tile([128, 4, T], bf16)
        waT_sb = const.tile([128, 4, 3 * C], bf16)
        for ct in range(4):
            eng = nc.sync if ct % 2 == 0 else nc.scalar
            eng.dma_start(xT_sb[:, ct, :], xT_d[:, ct, :])
            eng.dma_start(waT_sb[:, ct, :], waT_d[:, ct, :])
        gb_sb = const.tile([128, 4, 4], f32)
        nc.sync.dma_start(gb_sb[:], gbT.rearrange("(ct p) g -> p ct g", p=128))
        rT_sb = const.tile([128, 4, T], bf16)
        nc.scalar.dma_start(rT_sb[:], rT_rev.rearrange("(ct p) t -> p ct t", p=128))
        wpT_sb = const.tile([128, 4, C], bf16)
        nc.sync.dma_start(wpT_sb[:], w_projT.rearrange("(ct p) t -> p ct t", p=128))
        ident_b = const.tile([128, 128], bf16)
        make_identity(nc, ident_b[:, :])

        # bias rows (bf16) + a ones row: biases enter the q/k/proj PSUM
        # accumulations as K=1 rank-1 matmuls so no separate bias-add pass
        # is needed on the vector engines
        ba_row = const.tile([1, 3 * C], bf16)
        nc.sync.dma_start(ba_row[:], b_attn_bf[:, :])
        bv_sb = const.tile([128, 512], f32)
        nc.sync.dma_start(
            bv_sb[:], b_attn[:, 2 * 512 : 3 * 512].to_broadcast((128, 512))
        )
        bp_sb = const.tile([128, C], f32)
        nc.sync.dma_start(bp_sb[:], b_proj[:, :].to_broadcast((128, C)))
        ones_row = const.tile([1, 128], bf16)
        nc.gpsimd.memset(ones_row[:, :], 1.0)

        # v tiles: [t-block, head, 65]; col 64 is the ones column used to
        # accumulate the softmax denominator inside the AV matmul. Two sets
        # alternated per rep so the next rep's projection can start while the
        # previous rep's attention still reads v.
        v_sets = [
            [const.tile([128, 8, HD + 1], bf16, name=f"v{tt}_{i}") for tt in range(4)]
            for i in range(2)
        ]
        for vset in v_sets:
            for vt in vset:
                nc.gpsimd.memset(vt[:, :, HD : HD + 1], 1.0)

        # ev staging for the shear: double-buffered per qb, pads preloaded
        # with -1e30 once (the copy below never touches the pad region, so the
        # causal mask falls out of exp() for free)
        ev_bufs = []
        for qb in range(4):
            W = 128 * (qb + 1)
            pair = [const.tile([128, W + 128], bf16, name=f"ev{qb}_{i}") for i in range(2)]
            for t_ in pair:
                nc.gpsimd.memset(t_[:, W : W + 128], NEG)
            ev_bufs.append(pair)

        # deferred tail (y transpose + output projection) of rep r, issued
        # during rep r+1's stats/LN window so PE has work while DVE/ACT
        # compute layernorm statistics
        pending_y = [None]
        _tail = [None]

        def tail_stage(y_nat_prev):
            yT_sb = y_pool.tile([128, 4, T], bf16, name="yT")  # [c, t]
            for tt in range(4):
                tp4 = tp_ps.tile([128, 512], bf16, name="tp4_y", tag="tp")
                for ct in range(4):
                    nc.tensor.transpose(
                        tp4[:, ct * 128 : (ct + 1) * 128],
                        y_nat_prev[:, tt, ct * 128 : (ct + 1) * 128],
                        ident_b[:, :],
                    )
                nc.scalar.activation(
                    yT_sb[:, :, tt * 128 : (tt + 1) * 128],
                    tp4[:, :].rearrange("p (ct t) -> p ct t", t=128),
                    Act.Identity,
                )
            # output projection; bias folded into the DVE eviction
            for tt in range(4):
                ps = sc_ps.tile([128, 512], f32, name="proj_ps", tag="sc")
                for ct in range(4):
                    nc.tensor.matmul(
                        ps[:, :],
                        yT_sb[:, ct, tt * 128 : (tt + 1) * 128],
                        wpT_sb[:, ct, :],
                        start=(ct == 0),
                        stop=(ct == 3),
                    )
                ob = out_pool.tile([128, 512], f32, name="ob", tag="ob")
                nc.vector.scalar_tensor_tensor(
                    ob[:, :], ps[:, :], 1.0, bp_sb[:, :], Alu.mult, Alu.add,
                )
                nc.sync.dma_start(y[tt * 128 : (tt + 1) * 128, :], ob[:, :])

        _tail[0] = tail_stage

        i32 = mybir.dt.int32

        def newton_rsqrt(var4, sfx):
            # rsqrt of (var + EPS) on [128, 4]: bit-trick seed + 2 Newton
            # iterations, DVE-only so ACT keeps its single Exp LUT table
            vpe = stats.tile([128, 4], f32, name="vpe", tag=f"vpe{sfx}")
            nc.vector.tensor_scalar(vpe[:, :], var4, EPS, None, Alu.add)
            xa = stats.tile([128, 4], f32, name="xa", tag=f"xa{sfx}")
            xb = stats.tile([128, 4], f32, name="xb", tag=f"xb{sfx}")
            nc.vector.tensor_scalar(
                xb[:, :].bitcast(i32), vpe[:, :].bitcast(i32),
                1, None, Alu.logical_shift_right,
            )
            nc.vector.tensor_scalar(
                xa[:, :].bitcast(i32), xb[:, :].bitcast(i32),
                -1, 0x5F3759DF, Alu.mult, Alu.add,
            )
            cur, nxt = xa, xb
            for _it in range(2):
                n1 = stats.tile([128, 4], f32, name="n1", tag=f"n1_{sfx}{_it}")
                nc.vector.tensor_tensor(n1[:, :], cur[:, :], cur[:, :], Alu.mult)
                n2 = stats.tile([128, 4], f32, name="n2", tag=f"n2_{sfx}{_it}")
                nc.vector.scalar_tensor_tensor(
                    n2[:, :], n1[:, :], 1.0, vpe[:, :], Alu.mult, Alu.mult
                )
                n3 = stats.tile([128, 4], f32, name="n3", tag=f"n3_{sfx}{_it}")
                nc.vector.tensor_scalar(
                    n3[:, :], n2[:, :], -0.5, 1.5, Alu.mult, Alu.add
                )
                nc.vector.tensor_tensor(nxt[:, :], cur[:, :], n3[:, :], Alu.mult)
                cur, nxt = nxt, cur
            return cur

        for _rep in range(reps):
            v_sb = v_sets[_rep % 2]
            # ---- qkv projection + layernorm, stream-major -----------------
            # all of q first, then k, then v: q's stats/Newton/LN-apply chain
            # (DVE/ACT) overlaps k's matmuls on PE, and k's chain overlaps
            # v + the previous rep's deferred tail, so PE never starves.
            # q/k bias is folded into the PSUM accumulation as a K=1 rank-1
            # matmul (ones x bias row); bn_stats reads the PSUM directly so
            # stats and the (pure-copy, ACT) eviction run concurrently.
            qT_sb = qkT.tile([128, 4, T], bf16)
            kT_sb = qkT.tile([128, 4, T], bf16)
            ln_all = [[None] * 4, [None] * 4]
            rstds = [None, None]
            nmrs = [None, None]
            for s in range(3):
                nats = [None] * 4
                mv = (
                    stats.tile([128, 4, 2], f32, name="mv", tag=f"mv{s}")
                    if s < 2 else None
                )
                for tt in range(4):
                    ps = sc_ps.tile([128, 512], f32, name="qkv_ps", tag="sc")
                    for ct in range(4):
                        nc.tensor.matmul(
                            ps[:, :],
                            xT_sb[:, ct, tt * 128 : (tt + 1) * 128],
                            waT_sb[:, ct, s * 512 : (s + 1) * 512],
                            start=(ct == 0),
                            stop=(ct == 3 and s == 2),
                        )
                    if s < 2:
                        nc.tensor.matmul(
                            ps[:, :],
                            ones_row[:, 0:128],
                            ba_row[:, s * 512 : (s + 1) * 512],
                            start=False,
                            stop=True,
                        )
                        dst = qk_nat.tile(
                            [128, 512], f32, name="qn" if s == 0 else "kn",
                            tag="qn" if s == 0 else "kn",
                        )
                        st6 = stats.tile([128, 6], f32, name="st6", tag="st6")
                        nc.vector.bn_stats(st6[:, :], ps[:, :])
                        nc.vector.bn_aggr(mv[:, tt, :], st6[:, :])
                        nc.scalar.activation(dst[:, :], ps[:, :], Act.Identity)
                        nats[tt] = dst
                    else:
                        nc.vector.scalar_tensor_tensor(
                            v_sb[tt][:, :, 0:HD],
                            ps[:, :].rearrange("p (h d) -> p h d", d=HD),
                            1.0,
                            bv_sb[:, :].rearrange("p (h d) -> p h d", d=HD),
                            Alu.mult,
                            Alu.add,
                        )
                if s == 2:
                    break
                rstd = newton_rsqrt(mv[:, :, 1], s)
                nmr = stats.tile([128, 4], f32, name="nmr", tag=f"nmr{s}")
                nc.vector.scalar_tensor_tensor(
                    nmr[:, :], mv[:, :, 0], -1.0, rstd[:, :],
                    Alu.mult, Alu.mult,
                )
                rstds[s], nmrs[s] = rstd, nmr
                # LN applies split across ACT (activation scale/bias) and
                # DVE (tensor_scalar with per-partition scalar pointers)
                for tt in range(4):
                    ln = stats.tile(
                        [128, 512], bf16, name="ln", tag=f"ln{s}{tt}", bufs=2
                    )
                    if tt % 2 == 0:
                        nc.scalar.activation(
                            ln[:, :], nats[tt][:, :], Act.Identity,
                            bias=nmr[:, tt : tt + 1],
                            scale=rstd[:, tt : tt + 1],
                        )
                    else:
                        nc.vector.tensor_scalar(
                            ln[:, :], nats[tt][:, :],
                            rstd[:, tt : tt + 1], nmr[:, tt : tt + 1],
                            Alu.mult, Alu.add,
                        )
                    ln_all[s][tt] = ln

            # previous rep's y-transpose + projection: PE filler while the
            # k-stream layernorm chain drains on DVE/ACT
            if pending_y[0] is not None:
                tail_stage(pending_y[0])
                pending_y[0] = None

            y_nat = y_pool.tile([128, 4, C], bf16)  # [t, c], per (head, qb)

            def head_slices(h):
                ct_h = h // 2
                p0 = (h % 2) * 64
                q_h = qT_sb[:, ct_h, :][p0 : p0 + 64, :]
                k_h = kT_sb[:, ct_h, :][p0 : p0 + 64, :]
                r_h = rT_sb[:, ct_h, :][p0 : p0 + 64, :]
                return q_h, k_h, r_h

            def s2_stage(h):
                q_h, _, r_h = head_slices(h)
                rels = [None] * 4
                for qb in range(4):
                    W = 128 * (qb + 1)
                    # S2[i, u'] = q_i . R_{W-1-u'} (u reversed via host table)
                    s2 = s2_ps.tile([128, 512], f32, name="s2t", tag="s2")
                    nc.tensor.matmul(
                        s2[:, :W],
                        q_h[:, qb * 128 : (qb + 1) * 128],
                        r_h[:, T - W : T],
                        start=True,
                        stop=True,
                    )
                    ev = ev_bufs[qb][h % 2]
                    if qb == 0:
                        nc.scalar.activation(ev[:, 0:W], s2[:, :W], Act.Identity)
                    else:
                        nc.vector.tensor_copy(ev[:, 0:W], s2[:, :W])

                    # sheared read: rel[p, j] = ev[p, 127 - p + j]; issue cost
                    # split between the two otherwise-idle queues
                    rel = rel_pool.tile([128, W], bf16, name="rel", tag=f"rel{qb}")
                    L = ev.tensor.shape[-1]
                    src = bass.AP(ev.tensor, ev.offset + 127, [[L - 1, 128], [1, W]])
                    eng = nc.gpsimd if qb % 2 == 0 else nc.sync
                    eng.dma_start(rel[:, :], src)
                    rels[qb] = rel
                return rels

            def att_stage(h, rels):
                q_h, k_h, _ = head_slices(h)
                attT_all = attT_pool.tile([128, 4, T], bf16, name="attT", tag="attT")
                for qb in range(4):
                    # transposed scores: scT[k, q] = k.q + rel[q, k], per
                    # 128-wide k block; rel^T materialized by a second matmul
                    # (lhsT=rel block, rhs=identity) into the same PSUM region
                    sc = sc_ps.tile([128, 512], f32, name="sct", tag="sc")
                    for kb in range(qb + 1):
                        nc.tensor.matmul(
                            sc[:, kb * 128 : (kb + 1) * 128],
                            k_h[:, kb * 128 : (kb + 1) * 128],
                            q_h[:, qb * 128 : (qb + 1) * 128],
                            start=True,
                            stop=False,
                        )
                        nc.tensor.matmul(
                            sc[:, kb * 128 : (kb + 1) * 128],
                            rels[qb][:, kb * 128 : (kb + 1) * 128],
                            ident_b[:, :],
                            start=False,
                            stop=True,
                        )
                    nc.scalar.activation(
                        attT_all[:, 0 : qb + 1, qb * 128 : (qb + 1) * 128],
                        sc[:, : (qb + 1) * 128].rearrange("p (kb q) -> p kb q", q=128),
                        Act.Exp,
                        scale=0.125,
                    )

                # av[i, 0:64] = sum_k att v; av[i, 64] = softmax denominator
                av4 = av_ps.tile([128, 4, HD + 1], f32, name="av_ps", tag="av")
                for qb in range(4):
                    for jb in range(qb + 1):
                        nc.tensor.matmul(
                            av4[:, qb, :],
                            attT_all[:, jb, qb * 128 : (qb + 1) * 128],
                            v_sb[jb][:, h, :],
                            start=(jb == 0),
                            stop=(jb == qb),
                        )
                rec = stats.tile([128, 4], f32, name="rec", tag="rec")
                nc.vector.reciprocal(rec[:, :], av4[:, :, HD])
                nc.vector.tensor_tensor(
                    y_nat[:, :, h * HD : (h + 1) * HD],
                    av4[:, :, 0:HD],
                    rec[:, :].unsqueeze(2).to_broadcast((128, 4, HD)),
                    Alu.mult,
                )

            # transpose grouped per ct so gamma/beta (per-partition on the
            # transposed side) applies once per 512-wide row; ct-major and
            # s-interleaved, with heads 0/1's S2+shear pipelines launched as
            # soon as the ct0 blocks of qT land so the first rels are ready
            # when attention starts
            rels_q = []
            for ct in range(4):
                for s in range(2):
                    dstT = qT_sb if s == 0 else kT_sb
                    gsl = gb_sb[:, :, 2 * s : 2 * s + 1]
                    bsl = gb_sb[:, :, 2 * s + 1 : 2 * s + 2]
                    tpc = tp_ps.tile([128, 512], bf16, name="tp4_qk", tag="tp")
                    for tt in range(4):
                        nc.tensor.transpose(
                            tpc[:, tt * 128 : (tt + 1) * 128],
                            ln_all[s][tt][:, ct * 128 : (ct + 1) * 128],
                            ident_b[:, :],
                        )
                    if s == 0:
                        nc.scalar.activation(
                            dstT[:, ct, :],
                            tpc[:, :],
                            Act.Identity,
                            bias=bsl[:, ct, :], scale=gsl[:, ct, :],
                        )
                    else:
                        nc.vector.tensor_scalar(
                            dstT[:, ct, :], tpc[:, :],
                            gsl[:, ct, :], bsl[:, ct, :],
                            Alu.mult, Alu.add,
                        )
                if ct == 0:
                    rels_q = [s2_stage(0), s2_stage(1)]

            for h in range(NH):
                if h + 2 < NH:
                    rels_q.append(s2_stage(h + 2))
                att_stage(h, rels_q[0])
                rels_q.pop(0)

            pending_y[0] = y_nat
        if pending_y[0] is not None:
            _tail[0](pending_y[0])

    nc.compile()
    return nc


def _prep_maps(inputs):
    import ml_dtypes

    bf = ml_dtypes.bfloat16
    x = np.asarray(inputs["x"], np.float32)
    gbT = np.ascontiguousarray(
        np.stack(
            [
                np.asarray(inputs["q_gamma"], np.float32),
                np.asarray(inputs["q_beta"], np.float32),
                np.asarray(inputs["k_gamma"], np.float32),
                np.asarray(inputs["k_beta"], np.float32),
            ],
            axis=1,
        )
    )
    shared = {
        "w_attnT": np.ascontiguousarray(np.asarray(inputs["w_attn"], np.float32).T).astype(bf),
        "b_attn": np.asarray(inputs["b_attn"], np.float32).reshape(1, -1),
        "b_attn_bf": np.asarray(inputs["b_attn"], np.float32).reshape(1, -1).astype(bf),
        "b_proj_bf": np.asarray(inputs["b_proj"], np.float32).reshape(1, -1).astype(bf),
        "w_projT": np.ascontiguousarray(
            np.asarray(inputs["w_proj"], np.float32).T
        ).astype(bf),
        "b_proj": np.asarray(inputs["b_proj"], np.float32).reshape(1, -1),
        "gbT": gbT,
        "rT_rev": np.ascontiguousarray(
            np.asarray(inputs["rel_emb"], np.float32)[::-1].T
        ).astype(bf),
    }
    return [
        dict(shared, xT=np.ascontiguousarray(x[b].T).astype(bf))
        for b in range(N_CORES)
    ]


def kernel(**inputs):
    from concourse.bass_utils import run_bass_kernel_spmd

    if "nc" not in _CACHE:
        _CACHE["nc"] = _build_nc()
    nc = _CACHE["nc"]
    in_maps = _prep_maps(inputs)
    res = run_bass_kernel_spmd(nc, in_maps, core_ids=list(range(N_CORES)))
    return np.stack([res.results[b]["y"] for b in range(N_CORES)], axis=0)


# revision 51
# speedup vs baseline: 1.1484x; 1.1484x over previous
"""Causal self-attention with relative position bias, 8-way batch-parallel
across NeuronCores. Self-contained: takes full inputs, returns full output.

Per-core layout strategy:
  - host feeds transposed weights (c-major) so every matmul operand already
    has the contraction dim in partitions; no transpose loads on device
  - rel bias: S2 = q @ R_rev^T per head, then a sheared (diagonal)
    SBUF->SBUF DMA materializes rel[i, j] = S2[i, i-j]; pad columns pre-set
    to -1e30 make the causal mask fall out of exp() for free
  - scores computed TRANSPOSED per 128-block: scT[k, q] = k.q via swapped
    matmul operands; rel^T accumulated into the same PSUM region by a second
    matmul (lhsT=rel block, rhs=identity). exp() then writes att^T straight
    to SBUF -- no PE transposes of att, no vector-engine evictions of it
  - softmax denominator from a ones-column appended to v (row-sum rides the
    AV matmul); no max-subtraction (scores bounded well inside fp32 range)
  - q/k/proj biases enter their PSUM accumulations as K=1 rank-1 matmuls,
    so bn_stats reads the qkv PSUM directly and evictions are pure copies
  - qkv is stream-major (q, then k, then v) with per-stream DVE-only Newton
    rsqrt (single ACT LUT table): q's layernorm chain overlaps k's matmuls
  - the y-transpose + output projection of rep r are issued during rep
    r+1's layernorm window (PE filler); heads are software-pipelined 2 deep
    with the first two heads' S2/shear chains launched mid-layernorm
  - elementwise work is split across ACT (activation scale/bias) and DVE
    (tensor_scalar with per-partition scalar pointers); shear DMA issue cost
    alternates between the gpsimd and sync queues
  - bf16 throughout the matmul paths (validated ~4e-3 scale-relative error)
"""

import numpy as np

B, T, C, NH = 8, 512, 512, 8
HD = C // NH  # 64
N_CORES = 8
EPS = 1e-5
NEG = -1.0e30

_CACHE = {}


def _build_nc(reps: int = 1, pools=(2, 3, 2, 1), ln_bufs=3):
    from contextlib import ExitStack

    import concourse.bass as bass
    import concourse.mybir as mybir
    import concourse.tile as tile
    from concourse import bacc
    from concourse.masks import make_identity

    f32 = mybir.dt.float32
    bf16 = mybir.dt.bfloat16
    Alu = mybir.AluOpType
    Act = mybir.ActivationFunctionType

    nc = bacc.Bacc("TRN2")

    xT = nc.declare_dram_parameter("xT", [C, T], bf16, isOutput=False)
    w_attnT = nc.declare_dram_parameter("w_attnT", [C, 3 * C], bf16, isOutput=False)
    b_attn = nc.declare_dram_parameter("b_attn", [1, 3 * C], f32, isOutput=False)
    b_attn_bf = nc.declare_dram_parameter("b_attn_bf", [1, 3 * C], bf16, isOutput=False)
    b_proj_bf = nc.declare_dram_parameter("b_proj_bf", [1, C], bf16, isOutput=False)
    w_projT = nc.declare_dram_parameter("w_projT", [C, C], bf16, isOutput=False)
    b_proj = nc.declare_dram_parameter("b_proj", [1, C], f32, isOutput=False)
    gbT = nc.declare_dram_parameter("gbT", [C, 4], f32, isOutput=False)
    rT_rev = nc.declare_dram_parameter("rT_rev", [C, T], bf16, isOutput=False)
    y = nc.declare_dram_parameter("y", [T, C], f32, isOutput=True)

    with tile.TileContext(nc) as tc, ExitStack() as ctx:
        const = ctx.enter_context(tc.tile_pool(name="const", bufs=1))
        qk_nat = ctx.enter_context(tc.tile_pool(name="qk_nat", bufs=4))
        qkT = ctx.enter_context(tc.tile_pool(name="qkT", bufs=2))
        stats = ctx.enter_context(tc.tile_pool(name="stats", bufs=2))
        rel_pool = ctx.enter_context(tc.tile_pool(name="rel_pool", bufs=4))
        attT_pool = ctx.enter_context(tc.tile_pool(name="attT_pool", bufs=4))
        y_pool = ctx.enter_context(tc.tile_pool(name="y_pool", bufs=2))
        out_pool = ctx.enter_context(tc.tile_pool(name="out_pool", bufs=2))
        s2_ps = ctx.enter_context(tc.tile_pool(name="s2_ps", bufs=pools[0], space="PSUM"))
        sc_ps = ctx.enter_context(tc.tile_pool(name="sc_ps", bufs=pools[1], space="PSUM"))
        tp_ps = ctx.enter_context(tc.tile_pool(name="tp_ps", bufs=pools[2], space="PSUM"))
        av_ps = ctx.enter_context(tc.tile_pool(name="av_ps", bufs=pools[3], space="PSUM"))

        # ---- constant loads ------------------------------------------------
        xT_d = xT.rearrange("(ct p) t -> p ct t", p=128)
        waT_d = w_attnT.rearrange("(ct p) t -> p ct t", p=128)
        xT_sb = const.tile([128, 4, T], bf16)
        waT_sb = const.tile([128, 4, 3 * C], bf16)
        for ct in range(4):
            eng = nc.sync if ct % 2 == 0 else nc.scalar
            eng.dma_start(xT_sb[:, ct, :], xT_d[:, ct, :])
            eng.dma_start(waT_sb[:, ct, :], waT_d[:, ct, :])
        gb_sb = const.tile([128, 4, 4], f32)
        nc.sync.dma_start(gb_sb[:], gbT.rearrange("(ct p) g -> p ct g", p=128))
        rT_sb = const.tile([128, 4, T], bf16)
        nc.scalar.dma_start(rT_sb[:], rT_rev.rearrange("(ct p) t -> p ct t", p=128))
        wpT_sb = const.tile([128, 4, C], bf16)
        nc.sync.dma_start(wpT_sb[:], w_projT.rearrange("(ct p) t -> p ct t", p=128))
        ident_b = const.tile([128, 128], bf16)
        make_identity(nc, ident_b[:, :])

        # bias rows (bf16) + a ones row: biases enter the q/k/proj PSUM
        # accumulations as K=1 rank-1 matmuls so no separate bias-add pass
        # is needed on the vector engines
        ba_row = const.tile([1, 3 * C], bf16)
        nc.sync.dma_start(ba_row[:], b_attn_bf[:, :])
        bv_sb = const.tile([128, 512], f32)
        nc.sync.dma_start(
            bv_sb[:], b_attn[:, 2 * 512 : 3 * 512].to_broadcast((128, 512))
        )
        bp_sb = const.tile([128, C], f32)
        nc.sync.dma_start(bp_sb[:], b_proj[:, :].to_broadcast((128, C)))
        ones_row = const.tile([1, 128], bf16)
        nc.gpsimd.memset(ones_row[:, :], 1.0)

        # v tiles: [t-block, head, 65]; col 64 is the ones column used to
        # accumulate the softmax denominator inside the AV matmul. Two sets
        # alternated per rep so the next rep's projection can start while the
        # previous rep's attention still reads v.
        v_sets = [
            [const.tile([128, 8, HD + 1], bf16, name=f"v{tt}_{i}") for tt in range(4)]
            for i in range(2)
        ]
        for vset in v_sets:
            for vt in vset:
                nc.gpsimd.memset(vt[:, :, HD : HD + 1], 1.0)

        # ev staging for the shear: double-buffered per qb, pads preloaded
        # with -1e30 once (the copy below never touches the pad region, so the
        # causal mask falls out of exp() for free)
        ev_bufs = []
        for qb in range(4):
            W = 128 * (qb + 1)
            pair = [const.tile([128, W + 128], bf16, name=f"ev{qb}_{i}") for i in range(2)]
            for t_ in pair:
                nc.gpsimd.memset(t_[:, W : W + 128], NEG)
            ev_bufs.append(pair)

        # deferred tail (y transpose + output projection) of rep r, issued
        # during rep r+1's stats/LN window so PE has work while DVE/ACT
        # compute layernorm statistics
        pending_y = [None]
        _tail = [None]

        def tail_stage(y_nat_prev):
            yT_sb = y_pool.tile([128, 4, T], bf16, name="yT")  # [c, t]
            for tt in range(4):
                tp4 = tp_ps.tile([128, 512], bf16, name="tp4_y", tag="tp")
                for ct in range(4):
                    nc.tensor.transpose(
                        tp4[:, ct * 128 : (ct + 1) * 128],
                        y_nat_prev[:, tt, ct * 128 : (ct + 1) * 128],
                        ident_b[:, :],
                    )
                nc.scalar.activation(
                    yT_sb[:, :, tt * 128 : (tt + 1) * 128],
                    tp4[:, :].rearrange("p (ct t) -> p ct t", t=128),
                    Act.Identity,
                )
            # output projection; bias folded into the DVE eviction
            for tt in range(4):
                ps = sc_ps.tile([128, 512], f32, name="proj_ps", tag="sc")
                for ct in range(4):
                    nc.tensor.matmul(
                        ps[:, :],
                        yT_sb[:, ct, tt * 128 : (tt + 1) * 128],
                        wpT_sb[:, ct, :],
                        start=(ct == 0),
                        stop=(ct == 3),
                    )
                ob = out_pool.tile([128, 512], f32, name="ob", tag="ob")
                nc.vector.scalar_tensor_tensor(
                    ob[:, :], ps[:, :], 1.0, bp_sb[:, :], Alu.mult, Alu.add,
                )
                nc.sync.dma_start(y[tt * 128 : (tt + 1) * 128, :], ob[:, :])

        _tail[0] = tail_stage

        i32 = mybir.dt.int32

        def newton_rsqrt(var4, sfx):
            # rsqrt of (var + EPS) on [128, 4]: bit-trick seed + 2 Newton
            # iterations, DVE-only so ACT keeps its single Exp LUT table
            vpe = stats.tile([128, 4], f32, name="vpe", tag=f"vpe{sfx}")
            nc.vector.tensor_scalar(vpe[:, :], var4, EPS, None, Alu.add)
            xa = stats.tile([128, 4], f32, name="xa", tag=f"xa{sfx}")
            xb = stats.tile([128, 4], f32, name="xb", tag=f"xb{sfx}")
            nc.vector.tensor_scalar(
                xb[:, :].bitcast(i32), vpe[:, :].bitcast(i32),
                1, None, Alu.logical_shift_right,
            )
            nc.vector.tensor_scalar(
                xa[:, :].bitcast(i32), xb[:, :].bitcast(i32),
                -1, 0x5F3759DF, Alu.mult, Alu.add,
            )
            cur, nxt = xa, xb
            for _it in range(2):
                n1 = stats.tile([128, 4], f32, name="n1", tag=f"n1_{sfx}{_it}")
                nc.vector.tensor_tensor(n1[:, :], cur[:, :], cur[:, :], Alu.mult)
                n2 = stats.tile([128, 4], f32, name="n2", tag=f"n2_{sfx}{_it}")
                nc.vector.scalar_tensor_tensor(
                    n2[:, :], n1[:, :], 1.0, vpe[:, :], Alu.mult, Alu.mult
                )
                n3 = stats.tile([128, 4], f32, name="n3", tag=f"n3_{sfx}{_it}")
                nc.vector.tensor_scalar(
                    n3[:, :], n2[:, :], -0.5, 1.5, Alu.mult, Alu.add
                )
                nc.vector.tensor_tensor(nxt[:, :], cur[:, :], n3[:, :], Alu.mult)
                cur, nxt = nxt, cur
            return cur

        for _rep in range(reps):
            v_sb = v_sets[_rep % 2]
            # ---- qkv projection + layernorm, stream-major -----------------
            # all of q first, then k, then v: q's stats/Newton/LN-apply chain
            # (DVE/ACT) overlaps k's matmuls on PE, and k's chain overlaps
            # v + the previous rep's deferred tail, so PE never starves.
            # q/k bias is folded into the PSUM accumulation as a K=1 rank-1
            # matmul (ones x bias row); bn_stats reads the PSUM directly so
            # stats and the (pure-copy, ACT) eviction run concurrently.
            qT_sb = qkT.tile([128, 4, T], bf16)
            kT_sb = qkT.tile([128, 4, T], bf16)
            ln_all = [[None] * 4, [None] * 4]
            rstds = [None, None]
            nmrs = [None, None]
            for s in range(3):
                nats = [None] * 4
                mv = (
                    stats.tile([128, 4, 2], f32, name="mv", tag=f"mv{s}")
                    if s < 2 else None
                )
                for tt in range(4):
                    ps = sc_ps.tile([128, 512], f32, name="qkv_ps", tag="sc")
                    for ct in range(4):
                        nc.tensor.matmul(
                            ps[:, :],
                            xT_sb[:, ct, tt * 128 : (tt + 1) * 128],
                            waT_sb[:, ct, s * 512 : (s + 1) * 512],
                            start=(ct == 0),
                            stop=(ct == 3 and s == 2),
                        )
                    if s < 2:
                        nc.tensor.matmul(
                            ps[:, :],
                            ones_row[:, 0:128],
                            ba_row[:, s * 512 : (s + 1) * 512],
                            start=False,
                            stop=True,
                        )
                        dst = qk_nat.tile(
                            [128, 512], bf16, name="qn" if s == 0 else "kn",
                            tag="qn" if s == 0 else "kn",
                        )
                        nc.scalar.activation(dst[:, :], ps[:, :], Act.Identity)
                        st6 = stats.tile([128, 6], f32, name="st6", tag="st6")
                        nc.vector.bn_stats(st6[:, :], dst[:, :])
                        nc.vector.bn_aggr(mv[:, tt, :], st6[:, :])
                        nats[tt] = dst
                    else:
                        nc.vector.scalar_tensor_tensor(
                            v_sb[tt][:, :, 0:HD],
                            ps[:, :].rearrange("p (h d) -> p h d", d=HD),
                            1.0,
                            bv_sb[:, :].rearrange("p (h d) -> p h d", d=HD),
                            Alu.mult,
                            Alu.add,
                        )
                if s == 2:
                    break
                rstd = newton_rsqrt(mv[:, :, 1], s)
                nmr = stats.tile([128, 4], f32, name="nmr", tag=f"nmr{s}")
                nc.vector.scalar_tensor_tensor(
                    nmr[:, :], mv[:, :, 0], -1.0, rstd[:, :],
                    Alu.mult, Alu.mult,
                )
                rstds[s], nmrs[s] = rstd, nmr
                # LN applies split across ACT (activation scale/bias) and
                # DVE (tensor_scalar with per-partition scalar pointers)
                for tt in range(4):
                    ln = stats.tile(
                        [128, 512], bf16, name="ln", tag=f"ln{s}{tt}", bufs=2
                    )
                    if tt % 2 == 0:
                        nc.scalar.activation(
                            ln[:, :], nats[tt][:, :], Act.Identity,
                            bias=nmr[:, tt : tt + 1],
                            scale=rstd[:, tt : tt + 1],
                        )
                    else:
                        nc.vector.tensor_scalar(
                            ln[:, :], nats[tt][:, :],
                            rstd[:, tt : tt + 1], nmr[:, tt : tt + 1],
                            Alu.mult, Alu.add,
                        )
                    ln_all[s][tt] = ln

            # previous rep's y-transpose + projection: PE filler while the
            # k-stream layernorm chain drains on DVE/ACT
            if pending_y[0] is not None:
                tail_stage(pending_y[0])
                pending_y[0] = None

            y_nat = y_pool.tile([128, 4, C], bf16)  # [t, c], per (head, qb)

            def head_slices(h):
                ct_h = h // 2
                p0 = (h % 2) * 64
                q_h = qT_sb[:, ct_h, :][p0 : p0 + 64, :]
                k_h = kT_sb[:, ct_h, :][p0 : p0 + 64, :]
                r_h = rT_sb[:, ct_h, :][p0 : p0 + 64, :]
                return q_h, k_h, r_h

            def s2_stage(h):
                q_h, _, r_h = head_slices(h)
                rels = [None] * 4
                for qb in range(4):
                    W = 128 * (qb + 1)
                    # S2[i, u'] = q_i . R_{W-1-u'} (u reversed via host table)
                    s2 = s2_ps.tile([128, 512], f32, name="s2t", tag="s2")
                    nc.tensor.matmul(
                        s2[:, :W],
                        q_h[:, qb * 128 : (qb + 1) * 128],
                        r_h[:, T - W : T],
                        start=True,
                        stop=True,
                    )
                    ev = ev_bufs[qb][h % 2]
                    if qb == 0:
                        nc.scalar.activation(ev[:, 0:W], s2[:, :W], Act.Identity)
                    else:
                        nc.vector.tensor_copy(ev[:, 0:W], s2[:, :W])

                    # sheared read: rel[p, j] = ev[p, 127 - p + j]; issue cost
                    # split between the two otherwise-idle queues
                    rel = rel_pool.tile([128, W], bf16, name="rel", tag=f"rel{qb}")
                    L = ev.tensor.shape[-1]
                    src = bass.AP(ev.tensor, ev.offset + 127, [[L - 1, 128], [1, W]])
                    eng = nc.gpsimd if qb % 2 == 0 else nc.sync
                    eng.dma_start(rel[:, :], src)
                    rels[qb] = rel
                return rels

            def att_stage(h, rels):
                q_h, k_h, _ = head_slices(h)
                attT_all = attT_pool.tile([128, 4, T], bf16, name="attT", tag="attT")
                for qb in range(4):
                    # transposed scores: scT[k, q] = k.q + rel[q, k], per
                    # 128-wide k block; rel^T materialized by a second matmul
                    # (lhsT=rel block, rhs=identity) into the same PSUM region
                    sc = sc_ps.tile([128, 512], f32, name="sct", tag="sc")
                    for kb in range(qb + 1):
                        nc.tensor.matmul(
                            sc[:, kb * 128 : (kb + 1) * 128],
                            k_h[:, kb * 128 : (kb + 1) * 128],
                            q_h[:, qb * 128 : (qb + 1) * 128],
                            start=True,
                            stop=False,
                        )
                        nc.tensor.matmul(
                            sc[:, kb * 128 : (kb + 1) * 128],
                            rels[qb][:, kb * 128 : (kb + 1) * 128],
                            ident_b[:, :],
                            start=False,
                            stop=True,
                        )
                    nc.scalar.activation(
                        attT_all[:, 0 : qb + 1, qb * 128 : (qb + 1) * 128],
                        sc[:, : (qb + 1) * 128].rearrange("p (kb q) -> p kb q", q=128),
                        Act.Exp,
                        scale=0.125,
                    )

                # av[i, 0:64] = sum_k att v; av[i, 64] = softmax denominator
                av4 = av_ps.tile([128, 4, HD + 1], f32, name="av_ps", tag="av")
                for qb in range(4):
                    for jb in range(qb + 1):
                        nc.tensor.matmul(
                            av4[:, qb, :],
                            attT_all[:, jb, qb * 128 : (qb + 1) * 128],
                            v_sb[jb][:, h, :],
                            start=(jb == 0),
                            stop=(jb == qb),
                        )
                rec = stats.tile([128, 4], f32, name="rec", tag="rec")
                nc.vector.reciprocal(rec[:, :], av4[:, :, HD])
                nc.vector.tensor_tensor(
                    y_nat[:, :, h * HD : (h + 1) * HD],
                    av4[:, :, 0:HD],
                    rec[:, :].unsqueeze(2).to_broadcast((128, 4, HD)),
                    Alu.mult,
                )

            # transpose grouped per ct so gamma/beta (per-partition on the
            # transposed side) applies once per 512-wide row; ct-major and
            # s-interleaved, with heads 0/1's S2+shear pipelines launched as
            # soon as the ct0 blocks of qT land so the first rels are ready
            # when attention starts
            rels_q = []
            for ct in range(4):
                for s in range(2):
                    dstT = qT_sb if s == 0 else kT_sb
                    gsl = gb_sb[:, :, 2 * s : 2 * s + 1]
                    bsl = gb_sb[:, :, 2 * s + 1 : 2 * s + 2]
                    tpc = tp_ps.tile([128, 512], bf16, name="tp4_qk", tag="tp")
                    for tt in range(4):
                        nc.tensor.transpose(
                            tpc[:, tt * 128 : (tt + 1) * 128],
                            ln_all[s][tt][:, ct * 128 : (ct + 1) * 128],
                            ident_b[:, :],
                        )
                    if s == 0:
                        nc.scalar.activation(
                            dstT[:, ct, :],
                            tpc[:, :],
                            Act.Identity,
                            bias=bsl[:, ct, :], scale=gsl[:, ct, :],
                        )
                    else:
                        nc.vector.tensor_scalar(
                            dstT[:, ct, :], tpc[:, :],
                            gsl[:, ct, :], bsl[:, ct, :],
                            Alu.mult, Alu.add,
                        )
                if ct == 0:
                    rels_q = [s2_stage(0), s2_stage(1)]

            for h in range(NH):
                if h + 2 < NH:
                    rels_q.append(s2_stage(h + 2))
                att_stage(h, rels_q[0])
                rels_q.pop(0)

            pending_y[0] = y_nat
        if pending_y[0] is not None:
            _tail[0](pending_y[0])

    nc.compile()
    return nc


def _prep_maps(inputs):
    import ml_dtypes

    bf = ml_dtypes.bfloat16
    x = np.asarray(inputs["x"], np.float32)
    gbT = np.ascontiguousarray(
        np.stack(
            [
                np.asarray(inputs["q_gamma"], np.float32),
                np.asarray(inputs["q_beta"], np.float32),
                np.asarray(inputs["k_gamma"], np.float32),
                np.asarray(inputs["k_beta"], np.float32),
            ],
            axis=1,
        )
    )
    shared = {
        "w_attnT": np.ascontiguousarray(np.asarray(inputs["w_attn"], np.float32).T).astype(bf),
        "b_attn": np.asarray(inputs["b_attn"], np.float32).reshape(1, -1),
        "b_attn_bf": np.asarray(inputs["b_attn"], np.float32).reshape(1, -1).astype(bf),
        "b_proj_bf": np.asarray(inputs["b_proj"], np.float32).reshape(1, -1).astype(bf),
        "w_projT": np.ascontiguousarray(
            np.asarray(inputs["w_proj"], np.float32).T
        ).astype(bf),
        "b_proj": np.asarray(inputs["b_proj"], np.float32).reshape(1, -1),
        "gbT": gbT,
        "rT_rev": np.ascontiguousarray(
            np.asarray(inputs["rel_emb"], np.float32)[::-1].T
        ).astype(bf),
    }
    return [
        dict(shared, xT=np.ascontiguousarray(x[b].T).astype(bf))
        for b in range(N_CORES)
    ]


def kernel(**inputs):
    from concourse.bass_utils import run_bass_kernel_spmd

    if "nc" not in _CACHE:
        _CACHE["nc"] = _build_nc()
    nc = _CACHE["nc"]
    in_maps = _prep_maps(inputs)
    res = run_bass_kernel_spmd(nc, in_maps, core_ids=list(range(N_CORES)))
    return np.stack([res.results[b]["y"] for b in range(N_CORES)], axis=0)
